# revision 35
# baseline (speedup 1.0000x reference)
"""AdaptiveGlobalWeightedRankPooling2d on 8 Trainium2 NeuronCores.

Math: y[b,c] = sum_n sort_desc(x[b,c])[n] * w[c,n] / sum_n w[c,n]
with w[c,n] = sigmoid(dc_logit[c] ** n).  In f32, w[c,n] == 0.5 exactly
for n >= 18 (dc_logit ~ 0.4055) and |w-0.5| < 4e-8 for n >= 16, so

    y[b,c] = ( sum_{j<K} top_j * (w[c,j]-0.5)  +  0.5 * sum_n x[b,c,n] ) / sum_w[c]

with K=16: a top-16 selection + full row sum, not a sort.  x is staged to the
device as bf16 (rel-err contribution ~1.7e-3, well under the 2e-2 gate),
halving HBM traffic vs f32 — the memory-bound roofline for this problem.

Sharding: batch dim across 8 cores (4 batches/core), no collectives.

Per core: 1024 rows of N=16384 bf16, processed as 8 partition-tiles x 2
column segments (2MB bf16 units, 8-slot SBUF ring), raw bacc engine programs
with manual semaphores:
  - Sync/HWDGE: one dma_start per segment (tile 0's first segment is split
    into halves so compute starts earlier)
  - VectorE: 4-level tensor_tensor-max fold tree (bf16 runs in the 2x_1P
    DVE perf mode; plain max8 is 1x-only, so folding first halves the scan
    cost), then max8 on two 256-wide blocks -> 16 candidates/segment;
    f32-cast + merge of 32 (48 for tile 0) candidates -> top-16 (bf16
    max8/match_replace misbehave on small widths, so the merge runs in
    f32); weighted dot against host-precomputed rank weights.
    Also one scalar_tensor_tensor(add, accum_out) per odd segment: fused
    pair-add + f32 accumulate covers 2400 of each tile's row-sum elements
    to balance the ScalarE load.
  - ScalarE: remaining row sums via activation(Copy, accum_out).
DVE write acks are pipelined, so a consumer issued right after its producer
can read stale SBUF: every op is scheduled >= 2 ops after its producer
(two segments' fold chains + the previous tile's merge/dot interleave) and
carries a two-back vchain wait, which is pre-satisfied at issue.
Fold-tree candidate truncation + K=16 verified on the dataset on the host:
rel err 1.70e-3 (same as the bf16-quantization floor).
"""

import numpy as np

B, C, H, W = 32, 256, 128, 128
N = H * W                 # 16384
NCORES = 8
BS = B // NCORES          # 4 batches per core
ROWS = BS * C             # 1024 rows per core
P = 128                   # partitions
NTILES = ROWS // P        # 8
SEG = 8192                # bf16 elems per segment (2MB per [128, SEG] tile)
NSEG = N // SEG           # 2 segments per tile row
NSEGS = NTILES * NSEG     # 16 global segments
NSLOT = 8                 # SBUF ring depth
K = 16                    # top-K kept (|w-0.5| < 4e-8 beyond j=15)
NSUM = 4                  # sum slots per tile (tile 0 uses all 4)
RW = K + NSUM             # 20
ACT_SPLIT = 6492          # odd segment: ACT sums [0:6492], DVE STT the rest
DFOLD = 4096              # per tile>0: seg0[0:4096] pair-added by DMA CCE
STT_HALF = (SEG - ACT_SPLIT) // 2   # 1152
NEG_FILL = -3.0e38

_CACHE = {}


def _build():
    """Raw-bacc build: manual engine programs + semaphores."""
    if "nc_raw" in _CACHE:
        return _CACHE["nc_raw"]
    from concourse import bacc, mybir

    f32 = mybir.dt.float32
    bf16 = mybir.dt.bfloat16
    Copy = mybir.ActivationFunctionType.Copy
    X = mybir.AxisListType.X
    mult = mybir.AluOpType.mult
    add = mybir.AluOpType.add
    nc = bacc.Bacc(
        "TRN2", target_bir_lowering=False, debug=False, num_devices=NCORES
    )
    x = nc.dram_tensor("x", [ROWS, N], bf16, kind="ExternalInput").ap()
    # packed per-partition constants: [wu_half0 | wu_half1 | winv0 | winv1]
    cpk = nc.dram_tensor("cpk", [P, 2 * RW + 2], f32, kind="ExternalInput").ap()
    out = nc.dram_tensor("out", [P, NTILES], f32, kind="ExternalOutput").ap()
    import os
    dbg = bool(os.environ.get("KERNEL_DEBUG_RALL"))
    rall_out = (
        nc.dram_tensor("rall_out", [P, NTILES * RW], f32, kind="ExternalOutput").ap()
        if dbg
        else None
    )
    cand_out = (
        nc.dram_tensor("cand_out", [P, 48], f32, kind="ExternalOutput").ap()
        if dbg
        else None
    )
    fl4_out = (
        nc.dram_tensor("fl4_out", [P, 512], f32, kind="ExternalOutput").ap()
        if dbg
        else None
    )

    xbuf = nc.alloc_sbuf_tensor("xbuf", [P, NSLOT * SEG], bf16).ap()
    # two independent fold-buffer sets so the two segments' fold chains can
    # interleave (every DVE op >= 2 ops after its producer -> RAW waits are
    # pre-satisfied instead of stalling on the write-ack)
    fl1a = nc.alloc_sbuf_tensor("fl1a", [P, SEG // 2], bf16).ap()
    fl2a = nc.alloc_sbuf_tensor("fl2a", [P, SEG // 4], bf16).ap()
    fl3a = nc.alloc_sbuf_tensor("fl3a", [P, SEG // 8], bf16).ap()
    fl4a = nc.alloc_sbuf_tensor("fl4a", [P, SEG // 16], bf16).ap()
    fl5a = nc.alloc_sbuf_tensor("fl5a", [P, SEG // 32], bf16).ap()
    fl1b = nc.alloc_sbuf_tensor("fl1b", [P, SEG // 2], bf16).ap()
    fl2b = nc.alloc_sbuf_tensor("fl2b", [P, SEG // 4], bf16).ap()
    fl3b = nc.alloc_sbuf_tensor("fl3b", [P, SEG // 8], bf16).ap()
    fl4b = nc.alloc_sbuf_tensor("fl4b", [P, SEG // 16], bf16).ap()
    fl5b = nc.alloc_sbuf_tensor("fl5b", [P, SEG // 32], bf16).ap()
    sttd = nc.alloc_sbuf_tensor("sttd", [P, STT_HALF], bf16).ap()
    sfold = nc.alloc_sbuf_tensor("sfold", [P, DFOLD // 2], bf16).ap()
    cand = nc.alloc_sbuf_tensor("cand", [P, 48], bf16).ap()
    candf = nc.alloc_sbuf_tensor("candf", [P, 48], f32).ap()
    candf2 = nc.alloc_sbuf_tensor("candf2", [P, 48], f32).ap()
    rall = nc.alloc_sbuf_tensor("rall", [P, NTILES * RW], f32).ap()
    scr = nc.alloc_sbuf_tensor("scr", [P, RW], f32).ap()
    acc = nc.alloc_sbuf_tensor("acc", [P, 1], f32).ap()
    outsb = nc.alloc_sbuf_tensor("outsb", [P, NTILES], f32).ap()
    cpksb = nc.alloc_sbuf_tensor("cpksb", [P, 2 * RW + 2], f32).ap()
    wusb = cpksb[:, 0 : 2 * RW]
    winvsb = cpksb[:, 2 * RW : 2 * RW + 2]
    dummy = [
        nc.alloc_sbuf_tensor("actdummy0", [P, SEG], bf16).ap(),
        nc.alloc_sbuf_tensor("actdummy1", [P, SEG], bf16).ap(),
    ]
    fl4dbg = nc.alloc_sbuf_tensor("fl4dbg", [P, 512], f32).ap() if dbg else None

    seg_sem = [nc.alloc_semaphore(f"seg{k}") for k in range(NSLOT)]
    seg0a_sem = nc.alloc_semaphore("seg0a")  # first half of the very first fill
    gsem = nc.alloc_semaphore("gsem")        # CCE sum-fold DMA pairs
    cst_sem = nc.alloc_semaphore("cst")
    mset_sem = nc.alloc_semaphore("mset")
    out_sem = nc.alloc_semaphore("outd")
    vchain = nc.alloc_semaphore("vchain")
    achain = nc.alloc_semaphore("achain")

    # ---- static schedule bookkeeping (dry pass) -------------------------
    # vector emission:
    #   tile0 seg0 (two 4096-halves, serial): 10 ops
    #   tile0 seg1 (serial): [f1 stt f2 f3 f4 m8 m8] = 7 ops
    #   tiles 1..7 (batched, 20 ops): [f1a f1b stt cast' f2a m8' f2b mr'
    #     f3a m8' f3b mul' f4a red' f4b scale' m8 m8 m8 m8]  (' = previous
    #     tile's merge+dot interleaved as spacers)
    #   tail: tile7's merge+dot serial (7 ops)
    v_free = {}     # global seg -> vchain count once its xbuf slot reads done
    a_free = {}     # global seg -> achain count once its ACT slot reads done
    act_total = {}  # tile -> achain count once all its sums (incl sfold) done
    vcnt = 0
    acnt = 0
    for i in range(NSEGS):
        t, sg = divmod(i, NSEG)
        if i == 0:
            v_free[i] = vcnt + 7  # after f1B (op 7 of 12)
            vcnt += 12
            acnt += 2
        elif i == 1:
            v_free[i] = vcnt + 2  # after f1 + stt
            vcnt += 8
            acnt += 1
            act_total[0] = acnt
        elif sg == 0:
            v_free[i] = vcnt + 1      # f1a at batch pos 1
            v_free[i + 1] = vcnt + 3  # f1b + stt at batch pos 2,3
            vcnt += 22
            acnt += 1  # act_b: seg0[DFOLD:SEG]
        else:
            acnt += 1  # act_c: seg1[0:ACT_SPLIT]
        a_free[i] = acnt
        if sg == 1 and t >= 1:
            acnt += 1  # act_a: sfold (DMA-CCE folded seg0[0:DFOLD]), last
            act_total[t] = acnt
    vcnt += 7  # trailing merge+dot of the last tile
    V_TOTAL = vcnt
    A_TOTAL = acnt

    def seg_thresh(i):
        return 16 * (i // NSLOT + 1)

    def slot(i):
        return xbuf[:, (i % NSLOT) * SEG : (i % NSLOT + 1) * SEG]

    with nc.Block(no_gpsimd_drain=True) as block:

        @block.sync
        def _(sync):
            for i in range(NSEGS):
                t, sg = divmod(i, NSEG)
                if i >= NSLOT:
                    j = i - NSLOT
                    sync.wait_ge(vchain, v_free[j])
                    sync.wait_ge(achain, a_free[j])
                    if j % NSEG == 0 and j >= 2:
                        # folded seg0 slot: CCE pair also read it
                        sync.wait_ge(gsem, 32 * (j // NSEG))
                if i == 0:
                    sync.dma_start(
                        out=xbuf[:, 0 : SEG // 2],
                        in_=x[0:P, 0 : SEG // 2],
                    ).then_inc(seg0a_sem, 16)
                    sync.dma_start(
                        out=xbuf[:, SEG // 2 : SEG],
                        in_=x[0:P, SEG // 2 : SEG],
                    ).then_inc(seg_sem[0], 16)
                else:
                    sync.dma_start(
                        out=slot(i),
                        in_=x[t * P : (t + 1) * P, sg * SEG : (sg + 1) * SEG],
                    ).then_inc(seg_sem[i % NSLOT], 16)
            sync.wait_ge(vchain, V_TOTAL)
            sync.dma_start(out=out[:], in_=outsb[:]).then_inc(out_sem, 16)
            if dbg:
                sync.dma_start(out=rall_out[:], in_=rall[:]).then_inc(out_sem, 16)
                sync.dma_start(out=cand_out[:], in_=candf[:]).then_inc(out_sem, 16)
                sync.dma_start(out=fl4_out[:], in_=fl4dbg[:]).then_inc(out_sem, 16)
            sync.wait_ge(out_sem, 64 if dbg else 16)

        @block.gpsimd
        def _(g):
            # zero rall (no slot is unwritten anymore, but keep it cheap and
            # safe against NaN garbage), then drive the CCE sum-fold pairs:
            # per tile>0, sfold = seg0[0:2048] then sfold += seg0[2048:4096],
            # both as SWDGE SBUF->SBUF DMAs (SDMA inline adders; frees ACT).
            g.memset(rall[:], 0.0).then_inc(mset_sem, 1)
            hf = DFOLD // 2
            for t in range(1, NTILES):
                i0 = NSEG * t
                o0 = (i0 % NSLOT) * SEG
                g.wait_ge(seg_sem[i0 % NSLOT], seg_thresh(i0))
                # WAR: previous tile's sfold act must have drained
                if t >= 2:
                    g.wait_ge(achain, act_total[t - 1])
                g.dma_start(
                    out=sfold[:], in_=xbuf[:, o0 : o0 + hf]
                ).then_inc(gsem, 16)
                g.wait_ge(gsem, 32 * (t - 1) + 16)
                g.dma_start(
                    out=sfold[:],
                    in_=xbuf[:, o0 + hf : o0 + DFOLD],
                    accum_op=add,
                ).then_inc(gsem, 16)

        @block.scalar
        def _(s):
            s.dma_start(out=cpksb[:], in_=cpk[:]).then_inc(cst_sem, 16)
            s.wait_ge(mset_sem, 1)  # rall zeroed before any accum_out lands
            n = 0

            def act(src, col, wait_sem=None, wait_val=None):
                nonlocal n
                if wait_sem is not None:
                    s.wait_ge(wait_sem, wait_val)
                ins = s.activation(
                    dummy[n % 2][:, 0 : src.shape[1]],
                    src,
                    Copy,
                    bias=0.0,
                    scale=1.0,
                    accum_out=rall[:, col : col + 1],
                )
                if n >= 2:
                    ins._wait_ge(achain, n - 1)
                ins.then_inc(achain)
                n += 1

            for i in range(NSEGS):
                t, sg = divmod(i, NSEG)
                k = i % NSLOT
                off = k * SEG
                rb = t * RW
                if i == 0:
                    act(xbuf[:, 0 : SEG // 2], rb + K, seg0a_sem, 16)
                    act(xbuf[:, SEG // 2 : SEG], rb + K + 1, seg_sem[0], 16)
                elif i == 1:
                    act(xbuf[:, off : off + ACT_SPLIT], rb + K + 2,
                        seg_sem[k], seg_thresh(i))
                elif sg == 0:
                    # [0:DFOLD] is summed via the CCE sfold path
                    act(xbuf[:, off + DFOLD : off + SEG], rb + K + 1,
                        seg_sem[k], seg_thresh(i))
                else:
                    act(xbuf[:, off : off + ACT_SPLIT], rb + K + 2,
                        seg_sem[k], seg_thresh(i))
                    # sfold act last: gives the CCE pair time to complete
                    act(sfold[:], rb + K, gsem, 32 * t)
            assert n == A_TOTAL

        @block.vector
        def _(v):
            vc = 0

            def chain(ins, wait_at):
                nonlocal vc
                ins._wait_ge(vchain, wait_at)
                ins.then_inc(vchain)
                vc += 1
                return ins

            def serial(ins):
                # producer is the immediately preceding op: wait its write ack
                return chain(ins, vc)

            def spaced(ins):
                # producer is >= 2 ops back: the wait is already satisfied
                return chain(ins, vc - 1)

            def stt_op(off, rb):
                return v.scalar_tensor_tensor(
                    sttd[:],
                    xbuf[:, off + ACT_SPLIT : off + ACT_SPLIT + STT_HALF],
                    1.0,
                    xbuf[:, off + ACT_SPLIT + STT_HALF : off + SEG],
                    op0=mult,
                    op1=add,
                    accum_out=rall[:, rb + K + NSUM - 1 : rb + K + NSUM],
                )

            def merge_ops(t):
                # tile t's merge+dot as emit-thunks (interleaved by caller)
                rb = t * RW
                half = t % 2
                cw = 48 if t == 0 else 32
                ca = candf[:, 0:cw]
                cb = candf2[:, 0:cw]

                def op_mul(emit, t=t, rb=rb, half=half):
                    v.wait_ge(achain, act_total[t])
                    emit(
                        v.tensor_mul(
                            scr[:],
                            rall[:, rb : rb + RW],
                            wusb[:, half * RW : (half + 1) * RW],
                        )
                    )

                return [
                    lambda emit, ca=ca, cw=cw: emit(
                        v.tensor_copy(ca, cand[:, 0:cw])
                    ),
                    lambda emit, ca=ca, rb=rb: emit(v.max(rall[:, rb : rb + 8], ca)),
                    lambda emit, ca=ca, cb=cb, rb=rb: emit(
                        v.match_replace(cb, rall[:, rb : rb + 8], ca, NEG_FILL)
                    ),
                    lambda emit, cb=cb, rb=rb: emit(
                        v.max(rall[:, rb + 8 : rb + 16], cb)
                    ),
                    op_mul,
                    lambda emit: emit(v.reduce_sum(acc[:], scr[:], axis=X)),
                    lambda emit, t=t, half=half: emit(
                        v.tensor_scalar_mul(
                            outsb[:, t : t + 1], acc[:], winvsb[:, half : half + 1]
                        )
                    ),
                ]

            def foldtree(src, width, cand_off):
                # tile0 startup path: serial width-halving folds, 2x max8(128)
                cur = src
                w = width
                bufs = {2048: fl2a, 1024: fl3a, 512: fl4a, 256: fl5a}
                first = True
                while w > 256:
                    w //= 2
                    dst = bufs[w][:, 0:w]
                    ins = v.tensor_max(dst, cur[:, 0:w], cur[:, w : 2 * w])
                    chain(ins, vc if not first else max(0, vc - 1))
                    cur = bufs[w]
                    first = False
                serial(v.max(cand[:, cand_off : cand_off + 8], cur[:, 0:128]))
                serial(v.max(cand[:, cand_off + 8 : cand_off + 16], cur[:, 128:256]))

            v.wait_ge(cst_sem, 16)
            v.wait_ge(mset_sem, 1)

            # ---- tile 0, serial startup ----
            v.wait_ge(seg0a_sem, 16)
            foldtree(xbuf[:, 0 : SEG // 2], SEG // 2, 0)
            v.wait_ge(seg_sem[0], 16)
            foldtree(xbuf[:, SEG // 2 : SEG], SEG // 2, 16)
            v.wait_ge(seg_sem[1], seg_thresh(1))
            off1 = SEG  # slot 1
            w = SEG // 2
            spaced(
                v.tensor_max(
                    fl1b[:, 0:w], xbuf[:, off1 : off1 + w],
                    xbuf[:, off1 + w : off1 + SEG],
                )
            )
            spaced(stt_op(off1, 0))
            spaced(v.tensor_max(fl2b[:], fl1b[:, 0:2048], fl1b[:, 2048:4096]))
            serial(v.tensor_max(fl3b[:], fl2b[:, 0:1024], fl2b[:, 1024:2048]))
            serial(v.tensor_max(fl4b[:], fl3b[:, 0:512], fl3b[:, 512:1024]))
            serial(v.tensor_max(fl5b[:], fl4b[:, 0:256], fl4b[:, 256:512]))
            serial(v.max(cand[:, 32:40], fl5b[:, 0:128]))
            spaced(v.max(cand[:, 40:48], fl5b[:, 128:256]))
            pending = merge_ops(0)

            # ---- tiles 1..7, batched with prev merge interleaved ----
            for t in range(1, NTILES):
                i0, i1 = 2 * t, 2 * t + 1
                k0, k1 = i0 % NSLOT, i1 % NSLOT
                o0, o1 = k0 * SEG, k1 * SEG
                v.wait_ge(seg_sem[k0], seg_thresh(i0))
                v.wait_ge(seg_sem[k1], seg_thresh(i1))
                rb = t * RW
                w = SEG // 2
                mo = pending
                spaced(
                    v.tensor_max(
                        fl1a[:, 0:w], xbuf[:, o0 : o0 + w],
                        xbuf[:, o0 + w : o0 + SEG],
                    )
                )
                spaced(
                    v.tensor_max(
                        fl1b[:, 0:w], xbuf[:, o1 : o1 + w],
                        xbuf[:, o1 + w : o1 + SEG],
                    )
                )
                spaced(stt_op(o1, rb))
                mo[0](spaced)  # cast'
                spaced(v.tensor_max(fl2a[:], fl1a[:, 0:2048], fl1a[:, 2048:4096]))
                mo[1](spaced)  # m8'
                spaced(v.tensor_max(fl2b[:], fl1b[:, 0:2048], fl1b[:, 2048:4096]))
                mo[2](spaced)  # mr'
                spaced(v.tensor_max(fl3a[:], fl2a[:, 0:1024], fl2a[:, 1024:2048]))
                mo[3](spaced)  # m8'
                spaced(v.tensor_max(fl3b[:], fl2b[:, 0:1024], fl2b[:, 1024:2048]))
                mo[4](spaced)  # mul' (+achain wait)
                spaced(v.tensor_max(fl4a[:], fl3a[:, 0:512], fl3a[:, 512:1024]))
                mo[5](spaced)  # red'
                spaced(v.tensor_max(fl4b[:], fl3b[:, 0:512], fl3b[:, 512:1024]))
                mo[6](spaced)  # scale'
                spaced(v.tensor_max(fl5a[:], fl4a[:, 0:256], fl4a[:, 256:512]))
                spaced(v.tensor_max(fl5b[:], fl4b[:, 0:256], fl4b[:, 256:512]))
                spaced(v.max(cand[:, 0:8], fl5a[:, 0:128]))
                spaced(v.max(cand[:, 8:16], fl5a[:, 128:256]))
                spaced(v.max(cand[:, 16:24], fl5b[:, 0:128]))
                spaced(v.max(cand[:, 24:32], fl5b[:, 128:256]))
                pending = merge_ops(t)

            # ---- tail: last tile's merge+dot, serial ----
            for th in pending:
                th(serial)
            assert vc == V_TOTAL, (vc, V_TOTAL)

    nc.compile()
    _CACHE["nc_raw"] = nc
    return nc


def _host_weights(dc_logit: np.ndarray):
    """Per-channel rank-weight data, mirroring the reference's f32 weights.

    Computed in f64 then rounded to f32 (agrees with the reference's f32
    sigmoid(dc**j) to <=1 ulp where it differs from 0.5 at all).
    """
    dc = dc_logit.astype(np.float64)  # [C]
    j = np.arange(N, dtype=np.float64)
    pw = dc[:, None] ** j[None, :]  # [C, N]
    wfull = (1.0 / (1.0 + np.exp(-pw))).astype(np.float32)  # [C, N]
    dev = np.abs(wfull[:, K:] - np.float32(0.5))
    assert dev.max() < 1e-5, (
        f"top-{K} decomposition invalid: weight deviation {dev.max()} beyond K"
    )
    sum_w = wfull.astype(np.float64).sum(axis=1)  # [C]
    wu = np.empty((C, RW), np.float32)
    wu[:, :K] = wfull[:, :K] - np.float32(0.5)
    wu[:, K:] = np.float32(0.5)
    winv = (1.0 / sum_w).astype(np.float32)[:, None]  # [C, 1]
    return wu, winv


def _run_pjrt(nc, in_maps):
    """Like bass2jax.run_bass_via_pjrt's multi-core path, but pre-uploads
    all inputs to the devices (device_put + block) BEFORE dispatching the
    NEFF, so per-core execution windows don't overlap neighbors' input
    transfers (they share HBM stacks in pairs)."""
    import jax
    import numpy as np
    from jax.sharding import Mesh, NamedSharding, PartitionSpec
    from jax.experimental.shard_map import shard_map
    from concourse import bass2jax, mybir

    bass2jax.install_neuronx_cc_hook()
    assert nc.dbg_addr is None
    n_cores = len(in_maps)
    partition_name = (
        nc.partition_id_tensor.name if nc.partition_id_tensor else None
    )

    in_names, out_names, out_avals, zero_outs = [], [], [], []
    for alloc in nc.m.functions[0].allocations:
        if not isinstance(alloc, mybir.MemoryLocationSet):
            continue
        name = alloc.memorylocations[0].name
        if alloc.kind == "ExternalInput":
            if name != partition_name:
                in_names.append(name)
        elif alloc.kind == "ExternalOutput":
            shape = tuple(alloc.tensor_shape)
            dtype = mybir.dt.np(alloc.dtype)
            out_names.append(name)
            out_avals.append(jax.core.ShapedArray(shape, dtype))
            zero_outs.append(np.zeros(shape, dtype))
    n_params = len(in_names)
    n_outs = len(out_avals)
    all_in_names = list(in_names) + out_names
    if partition_name is not None:
        all_in_names.append(partition_name)
    donate = tuple(range(n_params, n_params + n_outs))

    def _body(*args):
        operands = list(args)
        if partition_name is not None:
            operands.append(bass2jax.partition_id_tensor())
        return tuple(
            bass2jax._bass_exec_p.bind(
                *operands,
                out_avals=tuple(out_avals),
                in_names=tuple(all_in_names),
                out_names=tuple(out_names),
                lowering_input_output_aliases=(),
                sim_require_finite=True,
                sim_require_nnan=True,
                nc=nc,
            )
        )

    devices = jax.devices()[:n_cores]
    mesh = Mesh(np.asarray(devices), ("core",))
    spec = PartitionSpec("core")
    sharded = jax.jit(
        shard_map(
            _body,
            mesh=mesh,
            in_specs=(spec,) * (n_params + n_outs),
            out_specs=(spec,) * n_outs,
            check_rep=False,
        ),
        donate_argnums=donate,
        keep_unused=True,
    )
    sh = NamedSharding(mesh, spec)
    concat_in = [
        jax.device_put(
            np.concatenate([np.asarray(in_maps[c][k]) for c in range(n_cores)], axis=0),
            sh,
        )
        for k in in_names
    ]
    concat_zeros = [
        jax.device_put(
            np.zeros((n_cores * z.shape[0], *z.shape[1:]), z.dtype), sh
        )
        for z in zero_outs
    ]
    jax.block_until_ready(concat_in)
    jax.block_until_ready(concat_zeros)
    out_arrs = sharded(*concat_in, *concat_zeros)
    return [
        {
            name: np.asarray(out_arrs[i]).reshape(n_cores, *out_avals[i].shape)[c]
            for i, name in enumerate(out_names)
        }
        for c in range(n_cores)
    ]


def _in_maps(x: np.ndarray, dc_logit: np.ndarray):
    import ml_dtypes

    wu, winv = _host_weights(np.asarray(dc_logit))
    cpk = np.empty((P, 2 * RW + 2), np.float32)
    cpk[:, 0:RW] = wu[0:P]
    cpk[:, RW : 2 * RW] = wu[P : 2 * P]
    cpk[:, 2 * RW] = winv[0:P, 0]
    cpk[:, 2 * RW + 1] = winv[P : 2 * P, 0]
    xr = np.ascontiguousarray(x).reshape(B * C, N).astype(ml_dtypes.bfloat16)
    return [
        {"x": xr[i * ROWS : (i + 1) * ROWS], "cpk": cpk}
        for i in range(NCORES)
    ]


def kernel(x: np.ndarray, dc_logit: np.ndarray) -> np.ndarray:
    import time

    nc = _build()
    in_maps = _in_maps(x, dc_logit)
    last_err = None
    for attempt in range(3):
        try:
            results = _run_pjrt(nc, in_maps)
            break
        except Exception as e:  # transient device errors (wedged core etc.)
            last_err = e
            time.sleep(15)
    else:
        raise last_err
    outs = []
    for i in range(NCORES):
        o = results[i]["out"]  # [P, NTILES]; col t, row p -> global row t*128+p
        outs.append(o.T.reshape(BS, C))
    return np.concatenate(outs, axis=0).astype(np.float32)


# revision 36
# speedup vs baseline: 1.1954x; 1.1954x over previous
"""AdaptiveGlobalWeightedRankPooling2d on 8 Trainium2 NeuronCores.

Math: y[b,c] = sum_n sort_desc(x[b,c])[n] * w[c,n] / sum_n w[c,n]
with w[c,n] = sigmoid(dc_logit[c] ** n).  In f32, w[c,n] == 0.5 exactly
for n >= 18 (dc_logit ~ 0.4055) and |w-0.5| < 4e-8 for n >= 16, so

    y[b,c] = ( sum_{j<K} top_j * (w[c,j]-0.5)  +  0.5 * sum_n x[b,c,n] ) / sum_w[c]

with K=16: a top-16 selection + full row sum, not a sort.  x is staged to the
device as bf16 (rel-err contribution ~1.7e-3, well under the 2e-2 gate),
halving HBM traffic vs f32 — the memory-bound roofline for this problem.

Sharding: batch dim across 8 cores (4 batches/core), no collectives.

Per core: 1024 rows of N=16384 bf16, processed as 8 partition-tiles x 2
column segments (2MB bf16 units, 8-slot SBUF ring), raw bacc engine programs
with manual semaphores:
  - Sync/HWDGE: one dma_start per segment (tile 0's first segment is split
    into halves so compute starts earlier)
  - VectorE: 4-level tensor_tensor-max fold tree (bf16 runs in the 2x_1P
    DVE perf mode; plain max8 is 1x-only, so folding first halves the scan
    cost), then max8 on two 256-wide blocks -> 16 candidates/segment;
    f32-cast + merge of 32 (48 for tile 0) candidates -> top-16 (bf16
    max8/match_replace misbehave on small widths, so the merge runs in
    f32); weighted dot against host-precomputed rank weights.
    Also one scalar_tensor_tensor(add, accum_out) per odd segment: fused
    pair-add + f32 accumulate covers 2400 of each tile's row-sum elements
    to balance the ScalarE load.
  - ScalarE: remaining row sums via activation(Copy, accum_out).
DVE write acks are pipelined, so a consumer issued right after its producer
can read stale SBUF: every op is scheduled >= 2 ops after its producer
(two segments' fold chains + the previous tile's merge/dot interleave) and
carries a two-back vchain wait, which is pre-satisfied at issue.
Fold-tree candidate truncation + K=16 verified on the dataset on the host:
rel err 1.70e-3 (same as the bf16-quantization floor).
"""

import numpy as np

B, C, H, W = 32, 256, 128, 128
N = H * W                 # 16384
NCORES = 8
BS = B // NCORES          # 4 batches per core
ROWS = BS * C             # 1024 rows per core
P = 128                   # partitions
NTILES = ROWS // P        # 8
SEG = 8192                # bf16 elems per segment (2MB per [128, SEG] tile)
NSEG = N // SEG           # 2 segments per tile row
NSEGS = NTILES * NSEG     # 16 global segments
NSLOT = 8                 # SBUF ring depth
K = 16                    # top-K kept (|w-0.5| < 4e-8 beyond j=15)
NSUM = 4                  # sum slots per tile (tile 0 uses all 4)
RW = K + NSUM             # 20
ACT_SPLIT = 5792          # odd segment: ACT sums [0:5792], DVE STT the rest
STT_HALF = (SEG - ACT_SPLIT) // 2   # 1152
NEG_FILL = -3.0e38

_CACHE = {}


def _build():
    """Raw-bacc build: manual engine programs + semaphores."""
    if "nc_raw" in _CACHE:
        return _CACHE["nc_raw"]
    from concourse import bacc, mybir

    f32 = mybir.dt.float32
    bf16 = mybir.dt.bfloat16
    Copy = mybir.ActivationFunctionType.Copy
    X = mybir.AxisListType.X
    mult = mybir.AluOpType.mult
    add = mybir.AluOpType.add
    nc = bacc.Bacc(
        "TRN2", target_bir_lowering=False, debug=False, num_devices=NCORES
    )
    x = nc.dram_tensor("x", [ROWS, N], bf16, kind="ExternalInput").ap()
    # packed per-partition constants: [wu_half0 | wu_half1 | winv0 | winv1]
    cpk = nc.dram_tensor("cpk", [P, 2 * RW + 2], f32, kind="ExternalInput").ap()
    out = nc.dram_tensor("out", [P, NTILES], f32, kind="ExternalOutput").ap()
    import os
    dbg = bool(os.environ.get("KERNEL_DEBUG_RALL"))
    rall_out = (
        nc.dram_tensor("rall_out", [P, NTILES * RW], f32, kind="ExternalOutput").ap()
        if dbg
        else None
    )
    cand_out = (
        nc.dram_tensor("cand_out", [P, 48], f32, kind="ExternalOutput").ap()
        if dbg
        else None
    )
    fl4_out = (
        nc.dram_tensor("fl4_out", [P, 512], f32, kind="ExternalOutput").ap()
        if dbg
        else None
    )

    xbuf = nc.alloc_sbuf_tensor("xbuf", [P, NSLOT * SEG], bf16).ap()
    # two independent fold-buffer sets so the two segments' fold chains can
    # interleave (every DVE op >= 2 ops after its producer -> RAW waits are
    # pre-satisfied instead of stalling on the write-ack)
    fl1a = nc.alloc_sbuf_tensor("fl1a", [P, SEG // 2], bf16).ap()
    fl2a = nc.alloc_sbuf_tensor("fl2a", [P, SEG // 4], bf16).ap()
    fl3a = nc.alloc_sbuf_tensor("fl3a", [P, SEG // 8], bf16).ap()
    fl4a = nc.alloc_sbuf_tensor("fl4a", [P, SEG // 16], bf16).ap()
    fl5a = nc.alloc_sbuf_tensor("fl5a", [P, SEG // 32], bf16).ap()
    fl1b = nc.alloc_sbuf_tensor("fl1b", [P, SEG // 2], bf16).ap()
    fl2b = nc.alloc_sbuf_tensor("fl2b", [P, SEG // 4], bf16).ap()
    fl3b = nc.alloc_sbuf_tensor("fl3b", [P, SEG // 8], bf16).ap()
    fl4b = nc.alloc_sbuf_tensor("fl4b", [P, SEG // 16], bf16).ap()
    fl5b = nc.alloc_sbuf_tensor("fl5b", [P, SEG // 32], bf16).ap()
    sttd = nc.alloc_sbuf_tensor("sttd", [P, STT_HALF], bf16).ap()
    cand = nc.alloc_sbuf_tensor("cand", [P, 48], bf16).ap()
    candf = nc.alloc_sbuf_tensor("candf", [P, 48], f32).ap()
    candf2 = nc.alloc_sbuf_tensor("candf2", [P, 48], f32).ap()
    rall = nc.alloc_sbuf_tensor("rall", [P, NTILES * RW], f32).ap()
    scr = nc.alloc_sbuf_tensor("scr", [P, RW], f32).ap()
    acc = nc.alloc_sbuf_tensor("acc", [P, 1], f32).ap()
    outsb = nc.alloc_sbuf_tensor("outsb", [P, NTILES], f32).ap()
    cpksb = nc.alloc_sbuf_tensor("cpksb", [P, 2 * RW + 2], f32).ap()
    wusb = cpksb[:, 0 : 2 * RW]
    winvsb = cpksb[:, 2 * RW : 2 * RW + 2]
    dummy = [
        nc.alloc_sbuf_tensor("actdummy0", [P, SEG], bf16).ap(),
        nc.alloc_sbuf_tensor("actdummy1", [P, SEG], bf16).ap(),
    ]
    fl4dbg = nc.alloc_sbuf_tensor("fl4dbg", [P, 512], f32).ap() if dbg else None

    seg_sem = [nc.alloc_semaphore(f"seg{k}") for k in range(NSLOT)]
    seg0a_sem = nc.alloc_semaphore("seg0a")  # first half of the very first fill
    cst_sem = nc.alloc_semaphore("cst")
    mset_sem = nc.alloc_semaphore("mset")
    out_sem = nc.alloc_semaphore("outd")
    vchain = nc.alloc_semaphore("vchain")
    achain = nc.alloc_semaphore("achain")

    # ---- static schedule bookkeeping (dry pass) -------------------------
    # vector emission:
    #   tile0 seg0 (two 4096-halves, serial): 10 ops
    #   tile0 seg1 (serial): [f1 stt f2 f3 f4 m8 m8] = 7 ops
    #   tiles 1..7 (batched, 20 ops): [f1a f1b stt cast' f2a m8' f2b mr'
    #     f3a m8' f3b mul' f4a red' f4b scale' m8 m8 m8 m8]  (' = previous
    #     tile's merge+dot interleaved as spacers)
    #   tail: tile7's merge+dot serial (7 ops)
    v_free = {}   # global seg -> vchain count once its xbuf slot reads done
    a_free = {}   # global seg -> achain count once its ACT slot reads done
    vcnt = 0
    acnt = 0
    for i in range(NSEGS):
        t, sg = divmod(i, NSEG)
        if i == 0:
            v_free[i] = vcnt + 7  # after f1B (op 7 of 12)
            vcnt += 12
            acnt += 2
        elif i == 1:
            v_free[i] = vcnt + 2  # after f1 + stt
            vcnt += 8
            acnt += 1
        elif sg == 0:
            v_free[i] = vcnt + 1      # f1a at batch pos 1
            v_free[i + 1] = vcnt + 3  # f1b + stt at batch pos 2,3
            vcnt += 22
            acnt += 1
        else:
            acnt += 1
        a_free[i] = acnt
    vcnt += 7  # trailing merge+dot of the last tile
    V_TOTAL = vcnt
    A_TOTAL = acnt

    def seg_thresh(i):
        return 16 * (i // NSLOT + 1)

    def slot(i):
        return xbuf[:, (i % NSLOT) * SEG : (i % NSLOT + 1) * SEG]

    with nc.Block(no_gpsimd_drain=True) as block:

        @block.sync
        def _(sync):
            for i in range(NSEGS):
                t, sg = divmod(i, NSEG)
                if i >= NSLOT:
                    j = i - NSLOT
                    sync.wait_ge(vchain, v_free[j])
                    sync.wait_ge(achain, a_free[j])
                if i == 0:
                    sync.dma_start(
                        out=xbuf[:, 0 : SEG // 2],
                        in_=x[0:P, 0 : SEG // 2],
                    ).then_inc(seg0a_sem, 16)
                    sync.dma_start(
                        out=xbuf[:, SEG // 2 : SEG],
                        in_=x[0:P, SEG // 2 : SEG],
                    ).then_inc(seg_sem[0], 16)
                else:
                    sync.dma_start(
                        out=slot(i),
                        in_=x[t * P : (t + 1) * P, sg * SEG : (sg + 1) * SEG],
                    ).then_inc(seg_sem[i % NSLOT], 16)
            sync.wait_ge(vchain, V_TOTAL)
            sync.dma_start(out=out[:], in_=outsb[:]).then_inc(out_sem, 16)
            if dbg:
                sync.dma_start(out=rall_out[:], in_=rall[:]).then_inc(out_sem, 16)
                sync.dma_start(out=cand_out[:], in_=candf[:]).then_inc(out_sem, 16)
                sync.dma_start(out=fl4_out[:], in_=fl4dbg[:]).then_inc(out_sem, 16)
            sync.wait_ge(out_sem, 64 if dbg else 16)

        @block.gpsimd
        def _(g):
            # zero rall: tiles > 0 leave their 4th sum slot unwritten
            g.memset(rall[:], 0.0).then_inc(mset_sem, 1)

        @block.scalar
        def _(s):
            s.dma_start(out=cpksb[:], in_=cpk[:]).then_inc(cst_sem, 16)
            s.wait_ge(mset_sem, 1)  # rall zeroed before any accum_out lands
            n = 0

            def act(src, col, wait_sem=None, wait_val=None):
                nonlocal n
                if wait_sem is not None:
                    s.wait_ge(wait_sem, wait_val)
                ins = s.activation(
                    dummy[n % 2][:, 0 : src.shape[1]],
                    src,
                    Copy,
                    bias=0.0,
                    scale=1.0,
                    accum_out=rall[:, col : col + 1],
                )
                if n >= 2:
                    ins._wait_ge(achain, n - 1)
                ins.then_inc(achain)
                n += 1

            for i in range(NSEGS):
                t, sg = divmod(i, NSEG)
                k = i % NSLOT
                rb = t * RW
                if i == 0:
                    act(xbuf[:, 0 : SEG // 2], rb + K, seg0a_sem, 16)
                    act(xbuf[:, SEG // 2 : SEG], rb + K + 1, seg_sem[0], 16)
                elif sg == 0:
                    act(slot(i), rb + K, seg_sem[k], seg_thresh(i))
                else:
                    off = (i % NSLOT) * SEG
                    act(
                        xbuf[:, off : off + ACT_SPLIT],
                        rb + K + (2 if t == 0 else 1),
                        seg_sem[k],
                        seg_thresh(i),
                    )
            assert n == A_TOTAL

        @block.vector
        def _(v):
            vc = 0

            def chain(ins, wait_at):
                nonlocal vc
                ins._wait_ge(vchain, wait_at)
                ins.then_inc(vchain)
                vc += 1
                return ins

            def serial(ins):
                # producer is the immediately preceding op: wait its write ack
                return chain(ins, vc)

            def spaced(ins):
                # producer is >= 2 ops back: the wait is already satisfied
                return chain(ins, vc - 1)

            def stt_op(off, rb):
                return v.scalar_tensor_tensor(
                    sttd[:],
                    xbuf[:, off + ACT_SPLIT : off + ACT_SPLIT + STT_HALF],
                    1.0,
                    xbuf[:, off + ACT_SPLIT + STT_HALF : off + SEG],
                    op0=mult,
                    op1=add,
                    accum_out=rall[:, rb + K + NSUM - 1 : rb + K + NSUM],
                )

            def merge_ops(t):
                # tile t's merge+dot as emit-thunks (interleaved by caller)
                rb = t * RW
                half = t % 2
                cw = 48 if t == 0 else 32
                ca = candf[:, 0:cw]
                cb = candf2[:, 0:cw]

                def op_mul(emit, t=t, rb=rb, half=half):
                    v.wait_ge(achain, a_free[2 * t + 1])
                    emit(
                        v.tensor_mul(
                            scr[:],
                            rall[:, rb : rb + RW],
                            wusb[:, half * RW : (half + 1) * RW],
                        )
                    )

                return [
                    lambda emit, ca=ca, cw=cw: emit(
                        v.tensor_copy(ca, cand[:, 0:cw])
                    ),
                    lambda emit, ca=ca, rb=rb: emit(v.max(rall[:, rb : rb + 8], ca)),
                    lambda emit, ca=ca, cb=cb, rb=rb: emit(
                        v.match_replace(cb, rall[:, rb : rb + 8], ca, NEG_FILL)
                    ),
                    lambda emit, cb=cb, rb=rb: emit(
                        v.max(rall[:, rb + 8 : rb + 16], cb)
                    ),
                    op_mul,
                    lambda emit: emit(v.reduce_sum(acc[:], scr[:], axis=X)),
                    lambda emit, t=t, half=half: emit(
                        v.tensor_scalar_mul(
                            outsb[:, t : t + 1], acc[:], winvsb[:, half : half + 1]
                        )
                    ),
                ]

            def foldtree(src, width, cand_off):
                # tile0 startup path: serial width-halving folds, 2x max8(128)
                cur = src
                w = width
                bufs = {2048: fl2a, 1024: fl3a, 512: fl4a, 256: fl5a}
                first = True
                while w > 256:
                    w //= 2
                    dst = bufs[w][:, 0:w]
                    ins = v.tensor_max(dst, cur[:, 0:w], cur[:, w : 2 * w])
                    chain(ins, vc if not first else max(0, vc - 1))
                    cur = bufs[w]
                    first = False
                serial(v.max(cand[:, cand_off : cand_off + 8], cur[:, 0:128]))
                serial(v.max(cand[:, cand_off + 8 : cand_off + 16], cur[:, 128:256]))

            v.wait_ge(cst_sem, 16)
            v.wait_ge(mset_sem, 1)

            # ---- tile 0, serial startup ----
            v.wait_ge(seg0a_sem, 16)
            foldtree(xbuf[:, 0 : SEG // 2], SEG // 2, 0)
            v.wait_ge(seg_sem[0], 16)
            foldtree(xbuf[:, SEG // 2 : SEG], SEG // 2, 16)
            v.wait_ge(seg_sem[1], seg_thresh(1))
            off1 = SEG  # slot 1
            w = SEG // 2
            spaced(
                v.tensor_max(
                    fl1b[:, 0:w], xbuf[:, off1 : off1 + w],
                    xbuf[:, off1 + w : off1 + SEG],
                )
            )
            spaced(stt_op(off1, 0))
            spaced(v.tensor_max(fl2b[:], fl1b[:, 0:2048], fl1b[:, 2048:4096]))
            serial(v.tensor_max(fl3b[:], fl2b[:, 0:1024], fl2b[:, 1024:2048]))
            serial(v.tensor_max(fl4b[:], fl3b[:, 0:512], fl3b[:, 512:1024]))
            serial(v.tensor_max(fl5b[:], fl4b[:, 0:256], fl4b[:, 256:512]))
            serial(v.max(cand[:, 32:40], fl5b[:, 0:128]))
            spaced(v.max(cand[:, 40:48], fl5b[:, 128:256]))
            pending = merge_ops(0)

            # ---- tiles 1..7, batched with prev merge interleaved ----
            for t in range(1, NTILES):
                i0, i1 = 2 * t, 2 * t + 1
                k0, k1 = i0 % NSLOT, i1 % NSLOT
                o0, o1 = k0 * SEG, k1 * SEG
                v.wait_ge(seg_sem[k0], seg_thresh(i0))
                v.wait_ge(seg_sem[k1], seg_thresh(i1))
                rb = t * RW
                w = SEG // 2
                mo = pending
                spaced(
                    v.tensor_max(
                        fl1a[:, 0:w], xbuf[:, o0 : o0 + w],
                        xbuf[:, o0 + w : o0 + SEG],
                    )
                )
                spaced(
                    v.tensor_max(
                        fl1b[:, 0:w], xbuf[:, o1 : o1 + w],
                        xbuf[:, o1 + w : o1 + SEG],
                    )
                )
                spaced(stt_op(o1, rb))
                mo[0](spaced)  # cast'
                spaced(v.tensor_max(fl2a[:], fl1a[:, 0:2048], fl1a[:, 2048:4096]))
                mo[1](spaced)  # m8'
                spaced(v.tensor_max(fl2b[:], fl1b[:, 0:2048], fl1b[:, 2048:4096]))
                mo[2](spaced)  # mr'
                spaced(v.tensor_max(fl3a[:], fl2a[:, 0:1024], fl2a[:, 1024:2048]))
                mo[3](spaced)  # m8'
                spaced(v.tensor_max(fl3b[:], fl2b[:, 0:1024], fl2b[:, 1024:2048]))
                mo[4](spaced)  # mul' (+achain wait)
                spaced(v.tensor_max(fl4a[:], fl3a[:, 0:512], fl3a[:, 512:1024]))
                mo[5](spaced)  # red'
                spaced(v.tensor_max(fl4b[:], fl3b[:, 0:512], fl3b[:, 512:1024]))
                mo[6](spaced)  # scale'
                spaced(v.tensor_max(fl5a[:], fl4a[:, 0:256], fl4a[:, 256:512]))
                spaced(v.tensor_max(fl5b[:], fl4b[:, 0:256], fl4b[:, 256:512]))
                spaced(v.max(cand[:, 0:8], fl5a[:, 0:128]))
                spaced(v.max(cand[:, 8:16], fl5a[:, 128:256]))
                spaced(v.max(cand[:, 16:24], fl5b[:, 0:128]))
                spaced(v.max(cand[:, 24:32], fl5b[:, 128:256]))
                pending = merge_ops(t)

            # ---- tail: last tile's merge+dot, serial ----
            for th in pending:
                th(serial)
            assert vc == V_TOTAL, (vc, V_TOTAL)

    nc.compile()
    _CACHE["nc_raw"] = nc
    return nc


def _host_weights(dc_logit: np.ndarray):
    """Per-channel rank-weight data, mirroring the reference's f32 weights.

    Computed in f64 then rounded to f32 (agrees with the reference's f32
    sigmoid(dc**j) to <=1 ulp where it differs from 0.5 at all).
    """
    dc = dc_logit.astype(np.float64)  # [C]
    j = np.arange(N, dtype=np.float64)
    pw = dc[:, None] ** j[None, :]  # [C, N]
    wfull = (1.0 / (1.0 + np.exp(-pw))).astype(np.float32)  # [C, N]
    dev = np.abs(wfull[:, K:] - np.float32(0.5))
    assert dev.max() < 1e-5, (
        f"top-{K} decomposition invalid: weight deviation {dev.max()} beyond K"
    )
    sum_w = wfull.astype(np.float64).sum(axis=1)  # [C]
    wu = np.empty((C, RW), np.float32)
    wu[:, :K] = wfull[:, :K] - np.float32(0.5)
    wu[:, K:] = np.float32(0.5)
    winv = (1.0 / sum_w).astype(np.float32)[:, None]  # [C, 1]
    return wu, winv


def _run_pjrt(nc, in_maps):
    """Like bass2jax.run_bass_via_pjrt's multi-core path, but pre-uploads
    all inputs to the devices (device_put + block) BEFORE dispatching the
    NEFF, so per-core execution windows don't overlap neighbors' input
    transfers (they share HBM stacks in pairs)."""
    import jax
    import numpy as np
    from jax.sharding import Mesh, NamedSharding, PartitionSpec
    from jax.experimental.shard_map import shard_map
    from concourse import bass2jax, mybir

    bass2jax.install_neuronx_cc_hook()
    assert nc.dbg_addr is None
    n_cores = len(in_maps)
    partition_name = (
        nc.partition_id_tensor.name if nc.partition_id_tensor else None
    )

    in_names, out_names, out_avals, zero_outs = [], [], [], []
    for alloc in nc.m.functions[0].allocations:
        if not isinstance(alloc, mybir.MemoryLocationSet):
            continue
        name = alloc.memorylocations[0].name
        if alloc.kind == "ExternalInput":
            if name != partition_name:
                in_names.append(name)
        elif alloc.kind == "ExternalOutput":
            shape = tuple(alloc.tensor_shape)
            dtype = mybir.dt.np(alloc.dtype)
            out_names.append(name)
            out_avals.append(jax.core.ShapedArray(shape, dtype))
            zero_outs.append(np.zeros(shape, dtype))
    n_params = len(in_names)
    n_outs = len(out_avals)
    all_in_names = list(in_names) + out_names
    if partition_name is not None:
        all_in_names.append(partition_name)
    donate = tuple(range(n_params, n_params + n_outs))

    def _body(*args):
        operands = list(args)
        if partition_name is not None:
            operands.append(bass2jax.partition_id_tensor())
        return tuple(
            bass2jax._bass_exec_p.bind(
                *operands,
                out_avals=tuple(out_avals),
                in_names=tuple(all_in_names),
                out_names=tuple(out_names),
                lowering_input_output_aliases=(),
                sim_require_finite=True,
                sim_require_nnan=True,
                nc=nc,
            )
        )

    devices = jax.devices()[:n_cores]
    mesh = Mesh(np.asarray(devices), ("core",))
    spec = PartitionSpec("core")
    sharded = jax.jit(
        shard_map(
            _body,
            mesh=mesh,
            in_specs=(spec,) * (n_params + n_outs),
            out_specs=(spec,) * n_outs,
            check_rep=False,
        ),
        donate_argnums=donate,
        keep_unused=True,
    )
    sh = NamedSharding(mesh, spec)
    concat_in = [
        jax.device_put(
            np.concatenate([np.asarray(in_maps[c][k]) for c in range(n_cores)], axis=0),
            sh,
        )
        for k in in_names
    ]
    concat_zeros = [
        jax.device_put(
            np.zeros((n_cores * z.shape[0], *z.shape[1:]), z.dtype), sh
        )
        for z in zero_outs
    ]
    jax.block_until_ready(concat_in)
    jax.block_until_ready(concat_zeros)
    out_arrs = sharded(*concat_in, *concat_zeros)
    return [
        {
            name: np.asarray(out_arrs[i]).reshape(n_cores, *out_avals[i].shape)[c]
            for i, name in enumerate(out_names)
        }
        for c in range(n_cores)
    ]


def _in_maps(x: np.ndarray, dc_logit: np.ndarray):
    import ml_dtypes

    wu, winv = _host_weights(np.asarray(dc_logit))
    cpk = np.empty((P, 2 * RW + 2), np.float32)
    cpk[:, 0:RW] = wu[0:P]
    cpk[:, RW : 2 * RW] = wu[P : 2 * P]
    cpk[:, 2 * RW] = winv[0:P, 0]
    cpk[:, 2 * RW + 1] = winv[P : 2 * P, 0]
    xr = np.ascontiguousarray(x).reshape(B * C, N).astype(ml_dtypes.bfloat16)
    return [
        {"x": xr[i * ROWS : (i + 1) * ROWS], "cpk": cpk}
        for i in range(NCORES)
    ]


def kernel(x: np.ndarray, dc_logit: np.ndarray) -> np.ndarray:
    import time

    nc = _build()
    in_maps = _in_maps(x, dc_logit)
    last_err = None
    for attempt in range(3):
        try:
            results = _run_pjrt(nc, in_maps)
            break
        except Exception as e:  # transient device errors (wedged core etc.)
            last_err = e
            time.sleep(15)
    else:
        raise last_err
    outs = []
    for i in range(NCORES):
        o = results[i]["out"]  # [P, NTILES]; col t, row p -> global row t*128+p
        outs.append(o.T.reshape(BS, C))
    return np.concatenate(outs, axis=0).astype(np.float32)


# revision 37
# speedup vs baseline: 1.3977x; 1.1692x over previous
"""AdaptiveGlobalWeightedRankPooling2d on 8 Trainium2 NeuronCores.

Math: y[b,c] = sum_n sort_desc(x[b,c])[n] * w[c,n] / sum_n w[c,n]
with w[c,n] = sigmoid(dc_logit[c] ** n).  In f32, w[c,n] == 0.5 exactly
for n >= 18 (dc_logit ~ 0.4055) and |w-0.5| < 4e-8 for n >= 16, so

    y[b,c] = ( sum_{j<K} top_j * (w[c,j]-0.5)  +  0.5 * sum_n x[b,c,n] ) / sum_w[c]

with K=16: a top-16 selection + full row sum, not a sort.  x is staged to the
device as bf16 (rel-err contribution ~1.7e-3, well under the 2e-2 gate),
halving HBM traffic vs f32 — the memory-bound roofline for this problem.

Sharding: batch dim across 8 cores (4 batches/core), no collectives.

Per core: 1024 rows of N=16384 bf16, processed as 8 partition-tiles x 2
column segments (2MB bf16 units, 8-slot SBUF ring), raw bacc engine programs
with manual semaphores:
  - Sync/HWDGE: one dma_start per segment (tile 0's first segment is split
    into halves so compute starts earlier)
  - VectorE: 4-level tensor_tensor-max fold tree (bf16 runs in the 2x_1P
    DVE perf mode; plain max8 is 1x-only, so folding first halves the scan
    cost), then max8 on two 256-wide blocks -> 16 candidates/segment;
    f32-cast + merge of 32 (48 for tile 0) candidates -> top-16 (bf16
    max8/match_replace misbehave on small widths, so the merge runs in
    f32); weighted dot against host-precomputed rank weights.
    Also one scalar_tensor_tensor(add, accum_out) per odd segment: fused
    pair-add + f32 accumulate covers 2400 of each tile's row-sum elements
    to balance the ScalarE load.
  - ScalarE: remaining row sums via activation(Copy, accum_out).
DVE write acks are pipelined, so a consumer issued right after its producer
can read stale SBUF: every op is scheduled >= 2 ops after its producer
(two segments' fold chains + the previous tile's merge/dot interleave) and
carries a two-back vchain wait, which is pre-satisfied at issue.
Fold-tree candidate truncation + K=16 verified on the dataset on the host:
rel err 1.70e-3 (same as the bf16-quantization floor).
"""

import numpy as np

B, C, H, W = 32, 256, 128, 128
N = H * W                 # 16384
NCORES = 8
BS = B // NCORES          # 4 batches per core
ROWS = BS * C             # 1024 rows per core
P = 128                   # partitions
NTILES = ROWS // P        # 8
SEG = 8192                # bf16 elems per segment (2MB per [128, SEG] tile)
NSEG = N // SEG           # 2 segments per tile row
NSEGS = NTILES * NSEG     # 16 global segments
NSLOT = 8                 # SBUF ring depth
K = 16                    # top-K kept (|w-0.5| < 4e-8 beyond j=15)
NSUM = 4                  # sum slots per tile (tile 0 uses all 4)
RW = K + NSUM             # 20
ACT_SPLIT = 5578          # odd segment: ACT sums [0:5578], DVE STT the rest
STT_HALF = (SEG - ACT_SPLIT) // 2   # 1152
NEG_FILL = -3.0e38

_CACHE = {}


def _build():
    """Raw-bacc build: manual engine programs + semaphores."""
    if "nc_raw" in _CACHE:
        return _CACHE["nc_raw"]
    from concourse import bacc, mybir

    f32 = mybir.dt.float32
    bf16 = mybir.dt.bfloat16
    Copy = mybir.ActivationFunctionType.Copy
    X = mybir.AxisListType.X
    mult = mybir.AluOpType.mult
    add = mybir.AluOpType.add
    nc = bacc.Bacc(
        "TRN2", target_bir_lowering=False, debug=False, num_devices=NCORES
    )
    x = nc.dram_tensor("x", [ROWS, N], bf16, kind="ExternalInput").ap()
    # packed per-partition constants: [wu_half0 | wu_half1 | winv0 | winv1]
    cpk = nc.dram_tensor("cpk", [P, 2 * RW + 2], f32, kind="ExternalInput").ap()
    out = nc.dram_tensor("out", [P, NTILES], f32, kind="ExternalOutput").ap()
    import os
    dbg = bool(os.environ.get("KERNEL_DEBUG_RALL"))
    rall_out = (
        nc.dram_tensor("rall_out", [P, NTILES * RW], f32, kind="ExternalOutput").ap()
        if dbg
        else None
    )
    cand_out = (
        nc.dram_tensor("cand_out", [P, 48], f32, kind="ExternalOutput").ap()
        if dbg
        else None
    )
    fl4_out = (
        nc.dram_tensor("fl4_out", [P, 512], f32, kind="ExternalOutput").ap()
        if dbg
        else None
    )

    xbuf = nc.alloc_sbuf_tensor("xbuf", [P, NSLOT * SEG], bf16).ap()
    # two independent fold-buffer sets so the two segments' fold chains can
    # interleave (every DVE op >= 2 ops after its producer -> RAW waits are
    # pre-satisfied instead of stalling on the write-ack)
    fl1a = nc.alloc_sbuf_tensor("fl1a", [P, SEG // 2], bf16).ap()
    fl2a = nc.alloc_sbuf_tensor("fl2a", [P, SEG // 4], bf16).ap()
    fl3a = nc.alloc_sbuf_tensor("fl3a", [P, SEG // 8], bf16).ap()
    fl4a = nc.alloc_sbuf_tensor("fl4a", [P, SEG // 16], bf16).ap()
    fl5a = nc.alloc_sbuf_tensor("fl5a", [P, SEG // 32], bf16).ap()
    fl1b = nc.alloc_sbuf_tensor("fl1b", [P, SEG // 2], bf16).ap()
    fl2b = nc.alloc_sbuf_tensor("fl2b", [P, SEG // 4], bf16).ap()
    fl3b = nc.alloc_sbuf_tensor("fl3b", [P, SEG // 8], bf16).ap()
    fl4b = nc.alloc_sbuf_tensor("fl4b", [P, SEG // 16], bf16).ap()
    fl5b = nc.alloc_sbuf_tensor("fl5b", [P, SEG // 32], bf16).ap()
    sttd = nc.alloc_sbuf_tensor("sttd", [P, STT_HALF], bf16).ap()
    cand = nc.alloc_sbuf_tensor("cand", [P, 48], bf16).ap()
    candf = nc.alloc_sbuf_tensor("candf", [P, 48], f32).ap()
    candf2 = nc.alloc_sbuf_tensor("candf2", [P, 48], f32).ap()
    rall = nc.alloc_sbuf_tensor("rall", [P, NTILES * RW], f32).ap()
    scr = nc.alloc_sbuf_tensor("scr", [P, RW], f32).ap()
    acc = nc.alloc_sbuf_tensor("acc", [P, 1], f32).ap()
    outsb = nc.alloc_sbuf_tensor("outsb", [P, NTILES], f32).ap()
    cpksb = nc.alloc_sbuf_tensor("cpksb", [P, 2 * RW + 2], f32).ap()
    wusb = cpksb[:, 0 : 2 * RW]
    winvsb = cpksb[:, 2 * RW : 2 * RW + 2]
    dummy = [
        nc.alloc_sbuf_tensor("actdummy0", [P, SEG], bf16).ap(),
        nc.alloc_sbuf_tensor("actdummy1", [P, SEG], bf16).ap(),
    ]
    fl4dbg = nc.alloc_sbuf_tensor("fl4dbg", [P, 512], f32).ap() if dbg else None

    seg_sem = [nc.alloc_semaphore(f"seg{k}") for k in range(NSLOT)]
    seg0a_sem = nc.alloc_semaphore("seg0a")  # first half of the very first fill
    cst_sem = nc.alloc_semaphore("cst")
    mset_sem = nc.alloc_semaphore("mset")
    out_sem = nc.alloc_semaphore("outd")
    vchain = nc.alloc_semaphore("vchain")
    achain = nc.alloc_semaphore("achain")

    # ---- static schedule bookkeeping (dry pass) -------------------------
    # vector emission:
    #   tile0 seg0 (two 4096-halves, serial): 10 ops
    #   tile0 seg1 (serial): [f1 stt f2 f3 f4 m8 m8] = 7 ops
    #   tiles 1..7 (batched, 20 ops): [f1a f1b stt cast' f2a m8' f2b mr'
    #     f3a m8' f3b mul' f4a red' f4b scale' m8 m8 m8 m8]  (' = previous
    #     tile's merge+dot interleaved as spacers)
    #   tail: tile7's merge+dot serial (7 ops)
    v_free = {}   # global seg -> vchain count once its xbuf slot reads done
    a_free = {}   # global seg -> achain count once its ACT slot reads done
    vcnt = 0
    acnt = 0
    for i in range(NSEGS):
        t, sg = divmod(i, NSEG)
        if i == 0:
            v_free[i] = vcnt + 7  # after f1B (op 7 of 12)
            vcnt += 12
            acnt += 2
        elif i == 1:
            v_free[i] = vcnt + 2  # after f1 + stt
            vcnt += 8
            acnt += 1
        elif sg == 0:
            v_free[i] = vcnt + 1      # f1a at batch pos 1
            v_free[i + 1] = vcnt + 3  # f1b + stt at batch pos 2,3
            vcnt += 22
            acnt += 1
        else:
            acnt += 1
        a_free[i] = acnt
    vcnt += 7  # trailing merge+dot of the last tile
    V_TOTAL = vcnt
    A_TOTAL = acnt

    def seg_thresh(i):
        return 16 * (i // NSLOT + 1)

    def slot(i):
        return xbuf[:, (i % NSLOT) * SEG : (i % NSLOT + 1) * SEG]

    with nc.Block(no_gpsimd_drain=True) as block:

        @block.sync
        def _(sync):
            for i in range(NSEGS):
                t, sg = divmod(i, NSEG)
                if i >= NSLOT:
                    j = i - NSLOT
                    sync.wait_ge(vchain, v_free[j])
                    sync.wait_ge(achain, a_free[j])
                if i == 0:
                    sync.dma_start(
                        out=xbuf[:, 0 : SEG // 2],
                        in_=x[0:P, 0 : SEG // 2],
                    ).then_inc(seg0a_sem, 16)
                    sync.dma_start(
                        out=xbuf[:, SEG // 2 : SEG],
                        in_=x[0:P, SEG // 2 : SEG],
                    ).then_inc(seg_sem[0], 16)
                else:
                    sync.dma_start(
                        out=slot(i),
                        in_=x[t * P : (t + 1) * P, sg * SEG : (sg + 1) * SEG],
                    ).then_inc(seg_sem[i % NSLOT], 16)
            sync.wait_ge(vchain, V_TOTAL)
            sync.dma_start(out=out[:], in_=outsb[:]).then_inc(out_sem, 16)
            if dbg:
                sync.dma_start(out=rall_out[:], in_=rall[:]).then_inc(out_sem, 16)
                sync.dma_start(out=cand_out[:], in_=candf[:]).then_inc(out_sem, 16)
                sync.dma_start(out=fl4_out[:], in_=fl4dbg[:]).then_inc(out_sem, 16)
            sync.wait_ge(out_sem, 64 if dbg else 16)

        @block.gpsimd
        def _(g):
            # zero rall: tiles > 0 leave their 4th sum slot unwritten
            g.memset(rall[:], 0.0).then_inc(mset_sem, 1)

        @block.scalar
        def _(s):
            s.dma_start(out=cpksb[:], in_=cpk[:]).then_inc(cst_sem, 16)
            s.wait_ge(mset_sem, 1)  # rall zeroed before any accum_out lands
            n = 0

            def act(src, col, wait_sem=None, wait_val=None):
                nonlocal n
                if wait_sem is not None:
                    s.wait_ge(wait_sem, wait_val)
                ins = s.activation(
                    dummy[n % 2][:, 0 : src.shape[1]],
                    src,
                    Copy,
                    bias=0.0,
                    scale=1.0,
                    accum_out=rall[:, col : col + 1],
                )
                if n >= 2:
                    ins._wait_ge(achain, n - 1)
                ins.then_inc(achain)
                n += 1

            for i in range(NSEGS):
                t, sg = divmod(i, NSEG)
                k = i % NSLOT
                rb = t * RW
                if i == 0:
                    act(xbuf[:, 0 : SEG // 2], rb + K, seg0a_sem, 16)
                    act(xbuf[:, SEG // 2 : SEG], rb + K + 1, seg_sem[0], 16)
                elif sg == 0:
                    act(slot(i), rb + K, seg_sem[k], seg_thresh(i))
                else:
                    off = (i % NSLOT) * SEG
                    act(
                        xbuf[:, off : off + ACT_SPLIT],
                        rb + K + (2 if t == 0 else 1),
                        seg_sem[k],
                        seg_thresh(i),
                    )
            assert n == A_TOTAL

        @block.vector
        def _(v):
            vc = 0

            def chain(ins, wait_at):
                nonlocal vc
                ins._wait_ge(vchain, wait_at)
                ins.then_inc(vchain)
                vc += 1
                return ins

            def serial(ins):
                # producer is the immediately preceding op: wait its write ack
                return chain(ins, vc)

            def spaced(ins):
                # producer is >= 2 ops back: the wait is already satisfied
                return chain(ins, vc - 1)

            def stt_op(off, rb):
                return v.scalar_tensor_tensor(
                    sttd[:],
                    xbuf[:, off + ACT_SPLIT : off + ACT_SPLIT + STT_HALF],
                    1.0,
                    xbuf[:, off + ACT_SPLIT + STT_HALF : off + SEG],
                    op0=mult,
                    op1=add,
                    accum_out=rall[:, rb + K + NSUM - 1 : rb + K + NSUM],
                )

            def merge_ops(t):
                # tile t's merge+dot as emit-thunks (interleaved by caller)
                rb = t * RW
                half = t % 2
                cw = 48 if t == 0 else 32
                ca = candf[:, 0:cw]
                cb = candf2[:, 0:cw]

                def op_mul(emit, t=t, rb=rb, half=half):
                    v.wait_ge(achain, a_free[2 * t + 1])
                    emit(
                        v.tensor_mul(
                            scr[:],
                            rall[:, rb : rb + RW],
                            wusb[:, half * RW : (half + 1) * RW],
                        )
                    )

                return [
                    lambda emit, ca=ca, cw=cw: emit(
                        v.tensor_copy(ca, cand[:, 0:cw])
                    ),
                    lambda emit, ca=ca, rb=rb: emit(v.max(rall[:, rb : rb + 8], ca)),
                    lambda emit, ca=ca, cb=cb, rb=rb: emit(
                        v.match_replace(cb, rall[:, rb : rb + 8], ca, NEG_FILL)
                    ),
                    lambda emit, cb=cb, rb=rb: emit(
                        v.max(rall[:, rb + 8 : rb + 16], cb)
                    ),
                    op_mul,
                    lambda emit: emit(v.reduce_sum(acc[:], scr[:], axis=X)),
                    lambda emit, t=t, half=half: emit(
                        v.tensor_scalar_mul(
                            outsb[:, t : t + 1], acc[:], winvsb[:, half : half + 1]
                        )
                    ),
                ]

            def foldtree(src, width, cand_off):
                # tile0 startup path: serial width-halving folds, 2x max8(128)
                cur = src
                w = width
                bufs = {2048: fl2a, 1024: fl3a, 512: fl4a, 256: fl5a}
                first = True
                while w > 256:
                    w //= 2
                    dst = bufs[w][:, 0:w]
                    ins = v.tensor_max(dst, cur[:, 0:w], cur[:, w : 2 * w])
                    chain(ins, vc if not first else max(0, vc - 1))
                    cur = bufs[w]
                    first = False
                serial(v.max(cand[:, cand_off : cand_off + 8], cur[:, 0:128]))
                serial(v.max(cand[:, cand_off + 8 : cand_off + 16], cur[:, 128:256]))

            v.wait_ge(cst_sem, 16)
            v.wait_ge(mset_sem, 1)

            # ---- tile 0, serial startup ----
            v.wait_ge(seg0a_sem, 16)
            foldtree(xbuf[:, 0 : SEG // 2], SEG // 2, 0)
            v.wait_ge(seg_sem[0], 16)
            foldtree(xbuf[:, SEG // 2 : SEG], SEG // 2, 16)
            v.wait_ge(seg_sem[1], seg_thresh(1))
            off1 = SEG  # slot 1
            w = SEG // 2
            spaced(
                v.tensor_max(
                    fl1b[:, 0:w], xbuf[:, off1 : off1 + w],
                    xbuf[:, off1 + w : off1 + SEG],
                )
            )
            spaced(stt_op(off1, 0))
            spaced(v.tensor_max(fl2b[:], fl1b[:, 0:2048], fl1b[:, 2048:4096]))
            serial(v.tensor_max(fl3b[:], fl2b[:, 0:1024], fl2b[:, 1024:2048]))
            serial(v.tensor_max(fl4b[:], fl3b[:, 0:512], fl3b[:, 512:1024]))
            serial(v.tensor_max(fl5b[:], fl4b[:, 0:256], fl4b[:, 256:512]))
            serial(v.max(cand[:, 32:40], fl5b[:, 0:128]))
            spaced(v.max(cand[:, 40:48], fl5b[:, 128:256]))
            pending = merge_ops(0)

            # ---- tiles 1..7, batched with prev merge interleaved ----
            for t in range(1, NTILES):
                i0, i1 = 2 * t, 2 * t + 1
                k0, k1 = i0 % NSLOT, i1 % NSLOT
                o0, o1 = k0 * SEG, k1 * SEG
                v.wait_ge(seg_sem[k0], seg_thresh(i0))
                v.wait_ge(seg_sem[k1], seg_thresh(i1))
                rb = t * RW
                w = SEG // 2
                mo = pending
                spaced(
                    v.tensor_max(
                        fl1a[:, 0:w], xbuf[:, o0 : o0 + w],
                        xbuf[:, o0 + w : o0 + SEG],
                    )
                )
                spaced(
                    v.tensor_max(
                        fl1b[:, 0:w], xbuf[:, o1 : o1 + w],
                        xbuf[:, o1 + w : o1 + SEG],
                    )
                )
                spaced(stt_op(o1, rb))
                mo[0](spaced)  # cast'
                spaced(v.tensor_max(fl2a[:], fl1a[:, 0:2048], fl1a[:, 2048:4096]))
                mo[1](spaced)  # m8'
                spaced(v.tensor_max(fl2b[:], fl1b[:, 0:2048], fl1b[:, 2048:4096]))
                mo[2](spaced)  # mr'
                spaced(v.tensor_max(fl3a[:], fl2a[:, 0:1024], fl2a[:, 1024:2048]))
                mo[3](spaced)  # m8'
                spaced(v.tensor_max(fl3b[:], fl2b[:, 0:1024], fl2b[:, 1024:2048]))
                mo[4](spaced)  # mul' (+achain wait)
                spaced(v.tensor_max(fl4a[:], fl3a[:, 0:512], fl3a[:, 512:1024]))
                mo[5](spaced)  # red'
                spaced(v.tensor_max(fl4b[:], fl3b[:, 0:512], fl3b[:, 512:1024]))
                mo[6](spaced)  # scale'
                spaced(v.tensor_max(fl5a[:], fl4a[:, 0:256], fl4a[:, 256:512]))
                spaced(v.tensor_max(fl5b[:], fl4b[:, 0:256], fl4b[:, 256:512]))
                spaced(v.max(cand[:, 0:8], fl5a[:, 0:128]))
                spaced(v.max(cand[:, 8:16], fl5a[:, 128:256]))
                spaced(v.max(cand[:, 16:24], fl5b[:, 0:128]))
                spaced(v.max(cand[:, 24:32], fl5b[:, 128:256]))
                pending = merge_ops(t)

            # ---- tail: last tile's merge+dot, serial ----
            for th in pending:
                th(serial)
            assert vc == V_TOTAL, (vc, V_TOTAL)

    nc.compile()
    _CACHE["nc_raw"] = nc
    return nc


def _host_weights(dc_logit: np.ndarray):
    """Per-channel rank-weight data, mirroring the reference's f32 weights.

    Computed in f64 then rounded to f32 (agrees with the reference's f32
    sigmoid(dc**j) to <=1 ulp where it differs from 0.5 at all).
    """
    dc = dc_logit.astype(np.float64)  # [C]
    j = np.arange(N, dtype=np.float64)
    pw = dc[:, None] ** j[None, :]  # [C, N]
    wfull = (1.0 / (1.0 + np.exp(-pw))).astype(np.float32)  # [C, N]
    dev = np.abs(wfull[:, K:] - np.float32(0.5))
    assert dev.max() < 1e-5, (
        f"top-{K} decomposition invalid: weight deviation {dev.max()} beyond K"
    )
    sum_w = wfull.astype(np.float64).sum(axis=1)  # [C]
    wu = np.empty((C, RW), np.float32)
    wu[:, :K] = wfull[:, :K] - np.float32(0.5)
    wu[:, K:] = np.float32(0.5)
    winv = (1.0 / sum_w).astype(np.float32)[:, None]  # [C, 1]
    return wu, winv


def _run_pjrt(nc, in_maps):
    """Like bass2jax.run_bass_via_pjrt's multi-core path, but pre-uploads
    all inputs to the devices (device_put + block) BEFORE dispatching the
    NEFF, so per-core execution windows don't overlap neighbors' input
    transfers (they share HBM stacks in pairs)."""
    import jax
    import numpy as np
    from jax.sharding import Mesh, NamedSharding, PartitionSpec
    from jax.experimental.shard_map import shard_map
    from concourse import bass2jax, mybir

    bass2jax.install_neuronx_cc_hook()
    assert nc.dbg_addr is None
    n_cores = len(in_maps)
    partition_name = (
        nc.partition_id_tensor.name if nc.partition_id_tensor else None
    )

    in_names, out_names, out_avals, zero_outs = [], [], [], []
    for alloc in nc.m.functions[0].allocations:
        if not isinstance(alloc, mybir.MemoryLocationSet):
            continue
        name = alloc.memorylocations[0].name
        if alloc.kind == "ExternalInput":
            if name != partition_name:
                in_names.append(name)
        elif alloc.kind == "ExternalOutput":
            shape = tuple(alloc.tensor_shape)
            dtype = mybir.dt.np(alloc.dtype)
            out_names.append(name)
            out_avals.append(jax.core.ShapedArray(shape, dtype))
            zero_outs.append(np.zeros(shape, dtype))
    n_params = len(in_names)
    n_outs = len(out_avals)
    all_in_names = list(in_names) + out_names
    if partition_name is not None:
        all_in_names.append(partition_name)
    donate = tuple(range(n_params, n_params + n_outs))

    def _body(*args):
        operands = list(args)
        if partition_name is not None:
            operands.append(bass2jax.partition_id_tensor())
        return tuple(
            bass2jax._bass_exec_p.bind(
                *operands,
                out_avals=tuple(out_avals),
                in_names=tuple(all_in_names),
                out_names=tuple(out_names),
                lowering_input_output_aliases=(),
                sim_require_finite=True,
                sim_require_nnan=True,
                nc=nc,
            )
        )

    devices = jax.devices()[:n_cores]
    mesh = Mesh(np.asarray(devices), ("core",))
    spec = PartitionSpec("core")
    sharded = jax.jit(
        shard_map(
            _body,
            mesh=mesh,
            in_specs=(spec,) * (n_params + n_outs),
            out_specs=(spec,) * n_outs,
            check_rep=False,
        ),
        donate_argnums=donate,
        keep_unused=True,
    )
    sh = NamedSharding(mesh, spec)
    concat_in = [
        jax.device_put(
            np.concatenate([np.asarray(in_maps[c][k]) for c in range(n_cores)], axis=0),
            sh,
        )
        for k in in_names
    ]
    concat_zeros = [
        jax.device_put(
            np.zeros((n_cores * z.shape[0], *z.shape[1:]), z.dtype), sh
        )
        for z in zero_outs
    ]
    jax.block_until_ready(concat_in)
    jax.block_until_ready(concat_zeros)
    out_arrs = sharded(*concat_in, *concat_zeros)
    return [
        {
            name: np.asarray(out_arrs[i]).reshape(n_cores, *out_avals[i].shape)[c]
            for i, name in enumerate(out_names)
        }
        for c in range(n_cores)
    ]


def _in_maps(x: np.ndarray, dc_logit: np.ndarray):
    import ml_dtypes

    wu, winv = _host_weights(np.asarray(dc_logit))
    cpk = np.empty((P, 2 * RW + 2), np.float32)
    cpk[:, 0:RW] = wu[0:P]
    cpk[:, RW : 2 * RW] = wu[P : 2 * P]
    cpk[:, 2 * RW] = winv[0:P, 0]
    cpk[:, 2 * RW + 1] = winv[P : 2 * P, 0]
    xr = np.ascontiguousarray(x).reshape(B * C, N).astype(ml_dtypes.bfloat16)
    return [
        {"x": xr[i * ROWS : (i + 1) * ROWS], "cpk": cpk}
        for i in range(NCORES)
    ]


def kernel(x: np.ndarray, dc_logit: np.ndarray) -> np.ndarray:
    import time

    nc = _build()
    in_maps = _in_maps(x, dc_logit)
    last_err = None
    for attempt in range(3):
        try:
            results = _run_pjrt(nc, in_maps)
            break
        except Exception as e:  # transient device errors (wedged core etc.)
            last_err = e
            time.sleep(15)
    else:
        raise last_err
    outs = []
    for i in range(NCORES):
        o = results[i]["out"]  # [P, NTILES]; col t, row p -> global row t*128+p
        outs.append(o.T.reshape(BS, C))
    return np.concatenate(outs, axis=0).astype(np.float32)
